# revision 16
# baseline (speedup 1.0000x reference)
"""Distributed FNO block on 8 TRN2 NeuronCores.

Strategy: batch-parallel forward/inverse (one batch per core), ky-sharded
spectral mode-mix (4 ky per core) with one AllToAll each direction.

The spectral path runs in fp8 (error contribution ~1e-5 of output norm; the
output is dominated by the bf16 channel-mixer y0):
 - forward DFTs use fp8 DoubleRow matmuls (K=256 in one instruction),
 - mode-mix uses fp8 weights (halves weight DMA),
 - inverse H-transform uses fp8 DoubleRow; the W-inverse and the y0 matmul
   accumulate into the same PSUM tile per output row (no y0 DRAM roundtrip,
   no vector adds).
Output is stored bf16 and widened to f32 on host.

Self-contained: shapes/sharding hardcoded, no sibling imports.
"""
import numpy as np
import ml_dtypes
from contextlib import ExitStack

import concourse.bass as bass
import concourse.bacc as bacc
import concourse.tile as tile
from concourse import mybir
from concourse.bass_utils import run_bass_kernel_spmd

B, C, H, W = 8, 128, 256, 256
M0, M1 = 32, 32
NCORES = 8
KX = np.concatenate([np.arange(32), np.arange(H - 32, H)])  # 64 kept kx modes
BF = mybir.dt.bfloat16
F32 = mybir.dt.float32
E4 = mybir.dt.float8e4
E5 = mybir.dt.float8e5
BF_NP = ml_dtypes.bfloat16
E4_NP = mybir.dt.np(E4)
E5_NP = mybir.dt.np(E5)

WSCALE = 8192.0          # wmix pre-scale (w in [0, 6.1e-5] -> [0, 0.5])
S2SCALE = 0.125          # stage2 = psm / 8
ZSCALE = 7.62939453125e-06  # zall = psZ / (2*256*256)


# ----------------------------------------------------------------- host consts
def _consts():
    # forward H-DFT (DoubleRow over ht): FH[hp, ht, kxri]
    FH = np.zeros((128, 2, 128), np.float32)
    for ht in range(2):
        h = ht * 128 + np.arange(128)
        th = 2 * np.pi * h[:, None] * KX[None, :] / H
        FH[:, ht, :64] = np.cos(th)
        FH[:, ht, 64:] = -np.sin(th)
    # forward W-DFT cos/sin halves (DoubleRow over wt), pre-scaled 1/4
    fwC = np.zeros((128, 2, 32), np.float32)
    fwS = np.zeros((128, 2, 32), np.float32)
    for wt in range(2):
        w = wt * 128 + np.arange(128)
        tw = 2 * np.pi * w[:, None] * np.arange(M1)[None, :] / W
        fwC[:, wt, :] = np.cos(tw) / 4
        fwS[:, wt, :] = np.sin(tw) / 4
    # inverse H (DoubleRow over rr): ghr -> Re(Z), ghi -> Im(Z), x H scale
    th = 2 * np.pi * np.arange(H)[None, :] * KX[:, None] / H  # [64kx, 256h]
    ghr = np.stack([np.cos(th), -np.sin(th)], axis=1)  # [64, 2, 256]
    ghi = np.stack([np.sin(th), np.cos(th)], axis=1)
    # inverse W (DoubleRow over zri); zall = Z_true/2 so cwp = 2*cw_true
    tw = 2 * np.pi * np.arange(M1)[:, None] * np.arange(W)[None, :] / W
    wt_ky = np.where(np.arange(M1) == 0, 1.0, 2.0)[:, None]
    cwp = np.stack([2 * wt_ky * np.cos(tw) / W,
                    -2 * wt_ky * np.sin(tw) / W], axis=1)  # [32, 2, 256]
    cwp[0, 1, :] = 0.0  # irfft drops Im(Z[ky=0])
    return (FH.astype(E4_NP), fwC.astype(E4_NP), fwS.astype(E4_NP),
            ghr.astype(E4_NP), ghi.astype(E4_NP), cwp.astype(E5_NP))


def _wmix_slices(w1r, w1i, w2r, w2i):
    """Per-core fp8 weight slice [4kyin, 4kxb, 128ci, 16kx, 2(wr,wi), 128co]."""
    wr = np.concatenate([w1r, w2r], axis=2)  # [ci, co, 64kx, 32ky]
    wi = np.concatenate([w1i, w2i], axis=2)
    wall = np.stack([wr, wi], axis=0) * WSCALE  # [2, ci, co, kx, ky]
    # -> [ky, kxb, ci, kxi, 2, co]
    wall = wall.reshape(2, C, C, 4, 16, 32).transpose(5, 3, 1, 4, 0, 2)
    wall = np.ascontiguousarray(wall).astype(E4_NP)
    return [np.ascontiguousarray(wall[4 * k:4 * k + 4]) for k in range(NCORES)]


# ----------------------------------------------------------------- bass kernel
def _build_nc():
    nc = bacc.Bacc(num_devices=NCORES)
    DR = mybir.MatmulPerfMode.DoubleRow

    x8_d = nc.declare_dram_parameter("x8", [C, 128, 2, W], E4, isOutput=False)
    xc_d = nc.declare_dram_parameter("xc", [C, H, W], BF, isOutput=False)
    wlt_d = nc.declare_dram_parameter("wlt", [C, C], BF, isOutput=False)
    fh_d = nc.declare_dram_parameter("fh", [128, 2, 128], E4, isOutput=False)
    fwc_d = nc.declare_dram_parameter("fwc", [128, 2, 32], E4, isOutput=False)
    fws_d = nc.declare_dram_parameter("fws", [128, 2, 32], E4, isOutput=False)
    ghr_d = nc.declare_dram_parameter("ghr", [64, 2, H], E4, isOutput=False)
    ghi_d = nc.declare_dram_parameter("ghi", [64, 2, H], E4, isOutput=False)
    cwp_d = nc.declare_dram_parameter("cwp", [32, 2, W], E5, isOutput=False)
    wmix_d = nc.declare_dram_parameter("wmix", [4, 4, C, 16, 2, C], E4,
                                       isOutput=False)
    out_d = nc.declare_dram_parameter("out", [C, H, W], BF, isOutput=True)

    # internal DRAM for the collectives (payload layouts chosen so the
    # receive side reshapes with a single 2-byte DMA transpose)
    send1 = nc.dram_tensor("send1", [8, 2, 4, 64, C], BF)  # [g,ri,ky,kx,c]
    recv1 = nc.dram_tensor("recv1", [8, 2, 4, 64, C], BF)
    send2 = nc.dram_tensor("send2", [8, 4, 2, C, 64], BF)  # [b,ky,rr,co,kx]
    recv2 = nc.dram_tensor("recv2", [8, 4, 2, C, 64], BF)

    rg = [list(range(NCORES))]

    with tile.TileContext(nc) as tc, ExitStack() as ctx:
        cpool = ctx.enter_context(tc.tile_pool(name="consts", bufs=1))
        spool = ctx.enter_context(tc.tile_pool(name="stages", bufs=1))
        xpool = ctx.enter_context(tc.tile_pool(name="x", bufs=8))
        xhpool = ctx.enter_context(tc.tile_pool(name="xh", bufs=6))
        wpool = ctx.enter_context(tc.tile_pool(name="wmix", bufs=3))
        xcpool = ctx.enter_context(tc.tile_pool(name="xc", bufs=8))
        opool = ctx.enter_context(tc.tile_pool(name="o", bufs=6))

        # constants
        fh_sb = cpool.tile([128, 2, 128], E4, tag="fh")
        fwc_sb = cpool.tile([128, 2, 32], E4, tag="fwc")
        fws_sb = cpool.tile([128, 2, 32], E4, tag="fws")
        ghr_sb = cpool.tile([64, 2, H], E4, tag="ghr")
        ghi_sb = cpool.tile([64, 2, H], E4, tag="ghi")
        cwp_sb = cpool.tile([32, 2, W], E5, tag="cwp")
        wlt_sb = cpool.tile([C, C], BF, tag="wlt")
        nc.sync.dma_start(fh_sb[:], fh_d[:])
        nc.sync.dma_start(fwc_sb[:], fwc_d[:])
        nc.sync.dma_start(fws_sb[:], fws_d[:])
        nc.sync.dma_start(ghr_sb[:], ghr_d[:])
        nc.sync.dma_start(ghi_sb[:], ghi_d[:])
        nc.sync.dma_start(cwp_sb[:], cwp_d[:])
        nc.sync.dma_start(wlt_sb[:], wlt_d[:])

        # big staging tiles
        stage1R = spool.tile([32, 64, C], BF, tag="s1r")   # [ky, kx, c]
        stage1I = spool.tile([32, 64, C], BF, tag="s1i")
        rhs1bf = spool.tile([C, 8, 2, 4, 64], BF, tag="r1b")  # [ci,b,ri,ky,kx]
        rhs1 = spool.tile([C, 8, 2, 4, 64], E4, tag="r1")
        rhs2 = spool.tile([C, 8, 2, 4, 64], E4, tag="r2")
        stage2 = spool.tile([C, 8, 4, 2, 64], BF, tag="s2")  # [co,b,ky,rr,kx]
        inv2bf = spool.tile([64, 8, 4, 2, C], BF, tag="i2b")  # [kx,g,kyin,rr,co]
        inv2 = spool.tile([64, 8, 4, 2, C], E4, tag="i2")
        zall = spool.tile([32, C, 2, H], E5, tag="zall")     # [ky,co,zri,h]

        # ---------------- forward truncated DFT (per channel) ----------------
        with tc.tile_pool(name="psA", bufs=3, space="PSUM") as psA_p, \
             tc.tile_pool(name="psCS", bufs=3, space="PSUM") as psCS_p:
            for c in range(C):
                xrow = xpool.tile([128, 2, W], E4, tag="xt")
                nc.sync.dma_start(xrow[:], x8_d[c])
                xh8 = xhpool.tile([128, 2, 128], E4, tag="xh")
                psA = psA_p.tile([128, 2, 128], F32, tag="psA")
                for wt in range(2):
                    nc.tensor.matmul(psA[:, wt, :],
                                     xrow[:, :, 128 * wt:128 * (wt + 1)],
                                     fh_sb[:], perf_mode=DR)
                nc.scalar.copy(xh8[:], psA[:])
                psCS = psCS_p.tile([32, 2, 128], F32, tag="psCS")
                nc.tensor.matmul(psCS[:, 0, :], fwc_sb[:], xh8[:], perf_mode=DR)
                nc.tensor.matmul(psCS[:, 1, :], fws_sb[:], xh8[:], perf_mode=DR)
                xs = xhpool.tile([32, 2, 128], BF, tag="xs")
                nc.scalar.copy(xs[:], psCS[:])
                # Xs_re = C[re] + S[im] ; Xs_im = C[im] - S[re]
                nc.vector.tensor_add(stage1R[:, :, c], xs[:, 0, 0:64],
                                     xs[:, 1, 64:128])
                nc.gpsimd.tensor_sub(stage1I[:, :, c], xs[:, 0, 64:128],
                                     xs[:, 1, 0:64])

        # A2A #1: ky-shard the spectrum (payload [g, ri, ky, kx, c])
        for g in range(8):
            nc.sync.dma_start(send1[g, 0], stage1R[4 * g:4 * g + 4, :, :])
            nc.sync.dma_start(send1[g, 1], stage1I[4 * g:4 * g + 4, :, :])
        nc.gpsimd.collective_compute(
            "AllToAll", mybir.AluOpType.bypass, replica_groups=rg,
            ins=[send1[:].opt()], outs=[recv1[:].opt()])

        # rhs build: one DMA transpose + dtype converts
        nc.sync.dma_start_transpose(
            rhs1bf[:].rearrange("c b r k x -> c (b r k x)"),
            recv1[:].rearrange("b r k x c -> (b r k x) c"))
        nc.vector.tensor_copy(rhs1[:], rhs1bf[:])
        nc.vector.tensor_scalar_mul(rhs2[:, :, 0, :, :],
                                    rhs1bf[:, :, 1, :, :], -1.0)
        nc.gpsimd.tensor_copy(rhs2[:, :, 1, :, :], rhs1bf[:, :, 0, :, :])

        # ---------------- modemix (ky-sharded, all batches) ------------------
        with tc.tile_pool(name="psm", bufs=4, space="PSUM") as psm_p:
            for kyi in range(4):
                for kxb in range(4):
                    wblk = wpool.tile([C, 16, 2, C], E4, tag="wblk")
                    nc.gpsimd.dma_start(wblk[:], wmix_d[kyi, kxb])
                    psm = psm_p.tile([C, 16, 2, 8], F32, tag="psm")
                    for kxi in range(16):
                        kx = 16 * kxb + kxi
                        nc.tensor.matmul(
                            psm[:, kxi, :, :], wblk[:, kxi, 0, :],
                            rhs1[:, :, :, kyi, kx].rearrange("c b r -> c r b"),
                            start=True, stop=False)
                        nc.tensor.matmul(
                            psm[:, kxi, :, :], wblk[:, kxi, 1, :],
                            rhs2[:, :, :, kyi, kx].rearrange("c b r -> c r b"),
                            start=False, stop=True)
                    # scaled copy into stage2 [co, b, kyi, rr, kx-block]
                    nc.vector.tensor_scalar_mul(
                        stage2[:, :, kyi, :, 16 * kxb:16 * (kxb + 1)]
                        .rearrange("c b r x -> c x r b"),
                        psm[:], S2SCALE)

        # A2A #2: back to batch-sharded spectrum (payload [b, ky, rr, co, kx])
        for b in range(8):
            nc.sync.dma_start(
                send2[b].rearrange("k r c x -> c k r x"), stage2[:, b])
        nc.gpsimd.collective_compute(
            "AllToAll", mybir.AluOpType.bypass, replica_groups=rg,
            ins=[send2[:].opt()], outs=[recv2[:].opt()])

        # inv2 [kx, g, kyin, rr, co]; ky-global = g*4 + kyin
        nc.sync.dma_start_transpose(
            inv2bf[:].rearrange("x g k r c -> x (g k r c)"),
            recv2[:].rearrange("g k r c x -> (g k r c) x"))
        nc.vector.tensor_copy(inv2[:, 0:4], inv2bf[:, 0:4])
        nc.gpsimd.tensor_copy(inv2[:, 4:8], inv2bf[:, 4:8])

        # ---------------- inverse H (fp8 DoubleRow over rr) ------------------
        with tc.tile_pool(name="psZ", bufs=4, space="PSUM") as psZ_p:
            for co in range(C):
                psZ = psZ_p.tile([32, 2, H], F32, tag="psZ")
                lhsT = inv2[:, :, :, :, co].rearrange("x g k r -> x r (g k)")
                nc.tensor.matmul(psZ[:, 0, :], lhsT, ghr_sb[:], perf_mode=DR)
                nc.tensor.matmul(psZ[:, 1, :], lhsT, ghi_sb[:], perf_mode=DR)
                nc.vector.tensor_scalar_mul(zall[:, co], psZ[:], ZSCALE)

        # ------------- inverse W + fused y0 (per output row h) ---------------
        with tc.tile_pool(name="psO", bufs=6, space="PSUM") as psO_p:
            for hb in range(64):
                xct = xcpool.tile([C, 4, W], BF, tag="xct")
                nc.scalar.dma_start(xct[:], xc_d[:, 4 * hb:4 * hb + 4, :])
                obuf = opool.tile([C, 4, W], BF, tag="obuf")
                for hq in range(4):
                    h = 4 * hb + hq
                    psO = psO_p.tile([C, W], F32, tag="psO")
                    nc.tensor.matmul(
                        psO[:], zall[:, :, :, h].rearrange("y c r -> y r c"),
                        cwp_sb[:], perf_mode=DR, start=True, stop=False)
                    nc.tensor.matmul(psO[:], wlt_sb[:], xct[:, hq, :],
                                     start=False, stop=True)
                    if hq % 2 == 0:
                        nc.vector.tensor_copy(obuf[:, hq, :], psO[:])
                    else:
                        nc.scalar.copy(obuf[:, hq, :], psO[:])
                nc.sync.dma_start(out_d[:, 4 * hb:4 * hb + 4, :], obuf[:])

    nc.compile()
    return nc


_NC_CACHE = {}


def kernel(x, W_lin, w1r, w1i, w2r, w2i):
    x = np.asarray(x)
    FH8, fwC8, fwS8, ghr8, ghi8, cwp8 = _consts()
    wlt = np.ascontiguousarray(np.asarray(W_lin).T).astype(BF_NP)
    wmix = _wmix_slices(np.asarray(w1r), np.asarray(w1i),
                        np.asarray(w2r), np.asarray(w2i))

    if "nc" not in _NC_CACHE:
        _NC_CACHE["nc"] = _build_nc()
    nc = _NC_CACHE["nc"]

    in_maps = []
    for k in range(NCORES):
        xk = np.ascontiguousarray(x[k])
        x8 = np.ascontiguousarray(
            xk.reshape(C, 2, 128, W).transpose(0, 2, 1, 3)).astype(E4_NP)
        in_maps.append({
            "x8": x8,
            "xc": xk.astype(BF_NP),
            "wlt": wlt,
            "fh": FH8, "fwc": fwC8, "fws": fwS8,
            "ghr": ghr8, "ghi": ghi8, "cwp": cwp8,
            "wmix": wmix[k],
        })
    res = run_bass_kernel_spmd(nc, in_maps, list(range(NCORES)))
    out = np.stack([res.results[k]["out"] for k in range(NCORES)], axis=0)
    return out.astype(np.float32)


# revision 23
# speedup vs baseline: 2.1208x; 2.1208x over previous
"""Distributed FNO block on 8 TRN2 NeuronCores.

Strategy: batch-parallel forward/inverse (one batch per core), ky-sharded
spectral mode-mix (4 ky per core) with one AllToAll each direction.

The spectral path runs in fp8 (error contribution ~1e-5 of output norm; the
output is dominated by the bf16 channel-mixer y0):
 - forward DFTs use fp8 DoubleRow matmuls (K=256 in one instruction),
 - mode-mix uses fp8 weights (halves weight DMA),
 - inverse H-transform uses fp8 DoubleRow; the W-inverse and the y0 matmul
   accumulate into the same PSUM tile per output row (no y0 DRAM roundtrip,
   no vector adds).
Output is stored bf16 and widened to f32 on host.

Self-contained: shapes/sharding hardcoded, no sibling imports.
"""
import numpy as np
import ml_dtypes
from contextlib import ExitStack

import concourse.bass as bass
import concourse.bacc as bacc
import concourse.tile as tile
from concourse import mybir
from concourse.bass_utils import run_bass_kernel_spmd

B, C, H, W = 8, 128, 256, 256
M0, M1 = 32, 32
NCORES = 8
KX = np.concatenate([np.arange(32), np.arange(H - 32, H)])  # 64 kept kx modes
BF = mybir.dt.bfloat16
F32 = mybir.dt.float32
E4 = mybir.dt.float8e4
E5 = mybir.dt.float8e5
BF_NP = ml_dtypes.bfloat16
E4_NP = mybir.dt.np(E4)
E5_NP = mybir.dt.np(E5)

WSCALE = 8192.0          # wmix pre-scale (w in [0, 6.1e-5] -> [0, 0.5])
S2SCALE = 0.125          # stage2 = psm / 8
ZSCALE = 7.62939453125e-06  # zall = psZ / (2*256*256)


# ----------------------------------------------------------------- host consts
def _consts():
    # forward H-DFT (DoubleRow over ht): FH[hp, ht, kxri]
    FH = np.zeros((128, 2, 128), np.float32)
    for ht in range(2):
        h = ht * 128 + np.arange(128)
        th = 2 * np.pi * h[:, None] * KX[None, :] / H
        FH[:, ht, :64] = np.cos(th)
        FH[:, ht, 64:] = -np.sin(th)
    # forward W-DFT cos/sin halves (DoubleRow over wt), pre-scaled 1/4
    fwC = np.zeros((128, 2, 32), np.float32)
    fwS = np.zeros((128, 2, 32), np.float32)
    for wt in range(2):
        w = wt * 128 + np.arange(128)
        tw = 2 * np.pi * w[:, None] * np.arange(M1)[None, :] / W
        fwC[:, wt, :] = np.cos(tw) / 4
        fwS[:, wt, :] = np.sin(tw) / 4
    # inverse H: partitions (rr, kx); ghr -> Re(Z), ghi -> Im(Z), x H scale
    th = 2 * np.pi * np.arange(H)[None, :] * KX[:, None] / H  # [64kx, 256h]
    ghr = np.concatenate([np.cos(th), -np.sin(th)], axis=0)  # [128, 256]
    ghi = np.concatenate([np.sin(th), np.cos(th)], axis=0)
    # inverse W (DoubleRow over zri); zall = Z_true/2 so cwp = 2*cw_true
    tw = 2 * np.pi * np.arange(M1)[:, None] * np.arange(W)[None, :] / W
    wt_ky = np.where(np.arange(M1) == 0, 1.0, 2.0)[:, None]
    cwp = np.stack([2 * wt_ky * np.cos(tw) / W,
                    -2 * wt_ky * np.sin(tw) / W], axis=1)  # [32, 2, 256]
    cwp[0, 1, :] = 0.0  # irfft drops Im(Z[ky=0])
    return (FH.astype(E4_NP), fwC.astype(E4_NP), fwS.astype(E4_NP),
            ghr.astype(E4_NP), ghi.astype(E4_NP), cwp.astype(E5_NP))


def _wmix_slices(w1r, w1i, w2r, w2i):
    """Per-core fp8 weight slice [4kyin, 4kxb, 128ci, 16kx, 2(wr,wi), 128co]."""
    wr = np.concatenate([w1r, w2r], axis=2)  # [ci, co, 64kx, 32ky]
    wi = np.concatenate([w1i, w2i], axis=2)
    wall = np.stack([wr, wi], axis=0) * WSCALE  # [2, ci, co, kx, ky]
    # -> [ky, kxb, ci, kxi, 2, co]
    wall = wall.reshape(2, C, C, 4, 16, 32).transpose(5, 3, 1, 4, 0, 2)
    wall = np.ascontiguousarray(wall).astype(E4_NP)
    return [np.ascontiguousarray(wall[4 * k:4 * k + 4]) for k in range(NCORES)]


# ----------------------------------------------------------------- bass kernel
def _build_nc():
    nc = bacc.Bacc(num_devices=NCORES)
    DR = mybir.MatmulPerfMode.DoubleRow

    x8_d = nc.declare_dram_parameter("x8", [C, 128, 2, W], E4, isOutput=False)
    xc_d = nc.declare_dram_parameter("xc", [C, H, W], BF, isOutput=False)
    wlt_d = nc.declare_dram_parameter("wlt", [C, C], BF, isOutput=False)
    fh_d = nc.declare_dram_parameter("fh", [128, 2, 128], E4, isOutput=False)
    fwc_d = nc.declare_dram_parameter("fwc", [128, 2, 32], E4, isOutput=False)
    fws_d = nc.declare_dram_parameter("fws", [128, 2, 32], E4, isOutput=False)
    ghr_d = nc.declare_dram_parameter("ghr", [128, H], E4, isOutput=False)
    ghi_d = nc.declare_dram_parameter("ghi", [128, H], E4, isOutput=False)
    cwp_d = nc.declare_dram_parameter("cwp", [32, 2, W], E5, isOutput=False)
    wmix_d = nc.declare_dram_parameter("wmix", [4, 4, C, 16, 2, C], E4,
                                       isOutput=False)
    out_d = nc.declare_dram_parameter("out", [C, H, W], BF, isOutput=True)

    # internal DRAM for the collectives (payload layouts chosen so the
    # receive side reshapes with a single 2-byte DMA transpose)
    send1 = nc.dram_tensor("send1", [8, 2, 4, 64, C], BF)  # [g,ri,ky,kx,c]
    recv1 = nc.dram_tensor("recv1", [8, 2, 4, 64, C], BF)
    send2 = nc.dram_tensor("send2", [8, 4, C, 2, 64], BF)  # [b,ky,co,rr,kx]
    recv2 = nc.dram_tensor("recv2", [8, 4, C, 2, 64], BF)

    rg = [list(range(NCORES))]

    with tile.TileContext(nc) as tc, ExitStack() as ctx:
        cpool = ctx.enter_context(tc.tile_pool(name="consts", bufs=1))
        spool = ctx.enter_context(tc.tile_pool(name="stages", bufs=1))
        xpool = ctx.enter_context(tc.tile_pool(name="x", bufs=8))
        xhpool = ctx.enter_context(tc.tile_pool(name="xh", bufs=6))
        wpool = ctx.enter_context(tc.tile_pool(name="wmix", bufs=3))
        xcpool = ctx.enter_context(tc.tile_pool(name="xc", bufs=8))
        opool = ctx.enter_context(tc.tile_pool(name="o", bufs=6))

        # constants
        fh_sb = cpool.tile([128, 2, 128], E4, tag="fh")
        fwc_sb = cpool.tile([128, 2, 32], E4, tag="fwc")
        fws_sb = cpool.tile([128, 2, 32], E4, tag="fws")
        ghr_sb = cpool.tile([128, H], E4, tag="ghr")
        ghi_sb = cpool.tile([128, H], E4, tag="ghi")
        cwp_sb = cpool.tile([32, 2, W], E5, tag="cwp")
        wlt_sb = cpool.tile([C, C], BF, tag="wlt")
        nc.sync.dma_start(fh_sb[:], fh_d[:])
        nc.sync.dma_start(fwc_sb[:], fwc_d[:])
        nc.sync.dma_start(fws_sb[:], fws_d[:])
        nc.sync.dma_start(ghr_sb[:], ghr_d[:])
        nc.sync.dma_start(ghi_sb[:], ghi_d[:])
        nc.sync.dma_start(cwp_sb[:], cwp_d[:])
        nc.sync.dma_start(wlt_sb[:], wlt_d[:])

        # big staging tiles
        stage1R = spool.tile([32, 64, C], BF, tag="s1r")   # [ky, kx, c]
        stage1I = spool.tile([32, 64, C], BF, tag="s1i")
        rhs1bf = spool.tile([C, 8, 2, 4, 64], BF, tag="r1b")  # [ci,b,ri,ky,kx]
        rhs1 = spool.tile([C, 8, 2, 4, 64], E4, tag="r1")
        rhs2 = spool.tile([C, 8, 2, 4, 64], E4, tag="r2")
        stage2 = spool.tile([C, 8, 4, 2, 64], BF, tag="s2")  # [co,b,ky,rr,kx]
        inv2bf = spool.tile([128, 8, 4, C], BF, tag="i2b")  # [(rr,kx),g,kyin,co]
        inv2 = spool.tile([128, 8, 4, C], E4, tag="i2")
        zall = spool.tile([32, C, 2, H], E5, tag="zall")     # [ky,co,zri,h]

        # ---------------- forward truncated DFT (per channel) ----------------
        with tc.tile_pool(name="psA", bufs=3, space="PSUM") as psA_p, \
             tc.tile_pool(name="psCS", bufs=3, space="PSUM") as psCS_p:
            for c in range(C):
                xrow = xpool.tile([128, 2, W], E4, tag="xt")
                nc.sync.dma_start(xrow[:], x8_d[c])
                xh8 = xhpool.tile([128, 2, 128], E4, tag="xh")
                psA = psA_p.tile([128, 2, 128], F32, tag="psA")
                for wt in range(2):
                    nc.tensor.matmul(psA[:, wt, :],
                                     xrow[:, :, 128 * wt:128 * (wt + 1)],
                                     fh_sb[:], perf_mode=DR)
                nc.scalar.copy(xh8[:], psA[:])
                psCS = psCS_p.tile([32, 2, 128], F32, tag="psCS")
                nc.tensor.matmul(psCS[:, 0, :], fwc_sb[:], xh8[:], perf_mode=DR)
                nc.tensor.matmul(psCS[:, 1, :], fws_sb[:], xh8[:], perf_mode=DR)
                xs = xhpool.tile([32, 2, 128], BF, tag="xs")
                nc.scalar.copy(xs[:], psCS[:])
                # Xs_re = C[re] + S[im] ; Xs_im = C[im] - S[re]
                nc.vector.tensor_add(stage1R[:, :, c], xs[:, 0, 0:64],
                                     xs[:, 1, 64:128])
                nc.gpsimd.tensor_sub(stage1I[:, :, c], xs[:, 0, 64:128],
                                     xs[:, 1, 0:64])

        # A2A #1: ky-shard the spectrum (payload [g, ri, ky, kx, c])
        for g in range(8):
            nc.sync.dma_start(send1[g, 0], stage1R[4 * g:4 * g + 4, :, :])
            nc.sync.dma_start(send1[g, 1], stage1I[4 * g:4 * g + 4, :, :])
        nc.gpsimd.collective_compute(
            "AllToAll", mybir.AluOpType.bypass, replica_groups=rg,
            ins=[send1[:].opt()], outs=[recv1[:].opt()])

        # rhs build: one DMA transpose + dtype converts
        nc.sync.dma_start_transpose(
            rhs1bf[:].rearrange("c b r k x -> c (b r k x)"),
            recv1[:].rearrange("b r k x c -> (b r k x) c"))
        nc.vector.tensor_copy(rhs1[:], rhs1bf[:])
        nc.vector.tensor_scalar_mul(rhs2[:, :, 0, :, :],
                                    rhs1bf[:, :, 1, :, :], -1.0)
        nc.gpsimd.tensor_copy(rhs2[:, :, 1, :, :], rhs1bf[:, :, 0, :, :])

        # ---------------- modemix (ky-sharded, all batches) ------------------
        with tc.tile_pool(name="psm", bufs=4, space="PSUM") as psm_p:
            for kyi in range(4):
                for kxb in range(4):
                    wblk = wpool.tile([C, 16, 2, C], E4, tag="wblk")
                    nc.gpsimd.dma_start(wblk[:], wmix_d[kyi, kxb])
                    psm = psm_p.tile([C, 16, 2, 8], F32, tag="psm")
                    for kxi in range(16):
                        kx = 16 * kxb + kxi
                        nc.tensor.matmul(
                            psm[:, kxi, :, :], wblk[:, kxi, 0, :],
                            rhs1[:, :, :, kyi, kx].rearrange("c b r -> c r b"),
                            start=True, stop=False)
                        nc.tensor.matmul(
                            psm[:, kxi, :, :], wblk[:, kxi, 1, :],
                            rhs2[:, :, :, kyi, kx].rearrange("c b r -> c r b"),
                            start=False, stop=True)
                    # scaled copy into stage2 [co, b, kyi, rr, kx-block]
                    nc.vector.tensor_scalar_mul(
                        stage2[:, :, kyi, :, 16 * kxb:16 * (kxb + 1)]
                        .rearrange("c b r x -> c x r b"),
                        psm[:], S2SCALE)

        # A2A #2: back to batch-sharded spectrum (payload [b, ky, co, rr, kx])
        for b in range(8):
            nc.sync.dma_start(
                send2[b].rearrange("k c r x -> c k r x"), stage2[:, b])
        nc.gpsimd.collective_compute(
            "AllToAll", mybir.AluOpType.bypass, replica_groups=rg,
            ins=[send2[:].opt()], outs=[recv2[:].opt()])

        # inv2 [(rr,kx), g, kyin, co]; ky-global = g*4 + kyin
        nc.sync.dma_start_transpose(
            inv2bf[:].rearrange("p g k c -> p (g k c)"),
            recv2[:].rearrange("g k c r x -> (g k c) (r x)"))
        nc.vector.tensor_copy(inv2[:, 0:4], inv2bf[:, 0:4])
        nc.gpsimd.tensor_copy(inv2[:, 4:8], inv2bf[:, 4:8])

        # ---------------- inverse H (fp8 DoubleRow over rr) ------------------
        with tc.tile_pool(name="psZ", bufs=4, space="PSUM") as psZ_p:
            for co in range(C):
                psZ = psZ_p.tile([32, 2, H], F32, tag="psZ")
                lhsT = inv2[:, :, :, co].rearrange("p g k -> p (g k)")
                nc.tensor.matmul(psZ[:, 0, :], lhsT, ghr_sb[:])
                nc.tensor.matmul(psZ[:, 1, :], lhsT, ghi_sb[:])
                nc.vector.tensor_scalar_mul(zall[:, co], psZ[:], ZSCALE)

        # ------------- inverse W + fused y0 (per output row h) ---------------
        with tc.tile_pool(name="psO", bufs=6, space="PSUM") as psO_p:
            for hb in range(64):
                xct = xcpool.tile([C, 4, W], BF, tag="xct")
                nc.scalar.dma_start(xct[:], xc_d[:, 4 * hb:4 * hb + 4, :])
                obuf = opool.tile([C, 4, W], BF, tag="obuf")
                for hq in range(4):
                    h = 4 * hb + hq
                    psO = psO_p.tile([C, W], F32, tag="psO")
                    nc.tensor.matmul(
                        psO[:], zall[:, :, :, h].rearrange("y c r -> y r c"),
                        cwp_sb[:], perf_mode=DR, start=True, stop=False)
                    nc.tensor.matmul(psO[:], wlt_sb[:], xct[:, hq, :],
                                     start=False, stop=True)
                    if hq % 2 == 0:
                        nc.vector.tensor_copy(obuf[:, hq, :], psO[:])
                    else:
                        nc.scalar.copy(obuf[:, hq, :], psO[:])
                nc.sync.dma_start(out_d[:, 4 * hb:4 * hb + 4, :], obuf[:])

    nc.compile()
    return nc


_NC_CACHE = {}


def kernel(x, W_lin, w1r, w1i, w2r, w2i):
    x = np.asarray(x)
    FH8, fwC8, fwS8, ghr8, ghi8, cwp8 = _consts()
    wlt = np.ascontiguousarray(np.asarray(W_lin).T).astype(BF_NP)
    wmix = _wmix_slices(np.asarray(w1r), np.asarray(w1i),
                        np.asarray(w2r), np.asarray(w2i))

    if "nc" not in _NC_CACHE:
        _NC_CACHE["nc"] = _build_nc()
    nc = _NC_CACHE["nc"]

    in_maps = []
    for k in range(NCORES):
        xk = np.ascontiguousarray(x[k])
        x8 = np.ascontiguousarray(
            xk.reshape(C, 2, 128, W).transpose(0, 2, 1, 3)).astype(E4_NP)
        in_maps.append({
            "x8": x8,
            "xc": xk.astype(BF_NP),
            "wlt": wlt,
            "fh": FH8, "fwc": fwC8, "fws": fwS8,
            "ghr": ghr8, "ghi": ghi8, "cwp": cwp8,
            "wmix": wmix[k],
        })
    res = run_bass_kernel_spmd(nc, in_maps, list(range(NCORES)))
    out = np.stack([res.results[k]["out"] for k in range(NCORES)], axis=0)
    return out.astype(np.float32)
